# revision 2
# baseline (speedup 1.0000x reference)
"""BinarizedDense TRN2 kernel: out = inputs @ (kernel > 0.5).

inputs [8192, 4096] f32, kernel [4096, 4096] f32 -> out [8192, 4096] f32.

Strategy
--------
Data parallel over tokens: each of the 8 NeuronCores computes a
[1024, 4096] output shard against the full binarized weight matrix.

Per core the matmul runs on the PE array in bf16 with an exact hi/lo
significand split of the activations: since the binarized weights are
exactly representable in bf16 ({0., 1.}), each product bf16(x)*b is
exact, so out = hi@B + lo@B accumulated in fp32 PSUM carries ~2^-17
relative representation error - on par with a plain fp32 matmul - while
streaming the PE at 1 cycle/row (4x the fp32 matmul rate).

Layout per core: activations are staged transposed ([in_f, tok], K on
partitions) and SBUF-resident (hi+lo = 128 KB/partition); the weight
matrix streams from HBM once as bf16 [128, 512] tiles; PSUM accumulates
over the 32 K-tiles; 8 token-tiles of one 512-wide output block occupy
the 8 PSUM banks; DVE evicts PSUM->SBUF and DMA writes the fp32 output.
"""
from contextlib import ExitStack

import numpy as np
import ml_dtypes

import concourse.bass as bass
import concourse.tile as tile
from concourse import bacc, mybir
from concourse.bass_utils import run_bass_kernel_spmd

TOKENS, IN_F, OUT_F = 8192, 4096, 4096
N_CORES = 8
TOK = TOKENS // N_CORES      # 1024 tokens per core
P = 128                      # partitions
NT = 512                     # output free-dim tile (one PSUM bank of fp32)
KT = IN_F // P               # 32 contraction tiles
MT = TOK // P                # 8 token tiles per core
NTI = OUT_F // NT            # 8 output blocks

_BF16 = mybir.dt.bfloat16
_F32 = mybir.dt.float32

_cached = None


def _build():
    nc = bacc.Bacc("TRN2", target_bir_lowering=False, debug=False)
    xhi = nc.dram_tensor("xhi", [IN_F, TOK], _BF16, kind="ExternalInput").ap()
    xlo = nc.dram_tensor("xlo", [IN_F, TOK], _BF16, kind="ExternalInput").ap()
    w = nc.dram_tensor("w", [IN_F, OUT_F], _BF16, kind="ExternalInput").ap()
    out = nc.dram_tensor("out", [TOK, OUT_F], _F32, kind="ExternalOutput").ap()

    with tile.TileContext(nc) as tc:
        with ExitStack() as ctx:
            xp = ctx.enter_context(tc.tile_pool(name="x", bufs=2 * KT))
            wp = ctx.enter_context(tc.tile_pool(name="w", bufs=8))
            op = ctx.enter_context(tc.tile_pool(name="o", bufs=8))
            pp = ctx.enter_context(tc.tile_pool(name="p", bufs=8, space="PSUM"))

            his, los = [], []
            for k in range(KT):
                th = xp.tile([P, TOK], _BF16, tag="x")
                nc.sync.dma_start(th[:], xhi[k * P:(k + 1) * P, :])
                his.append(th)
                tl = xp.tile([P, TOK], _BF16, tag="x")
                nc.sync.dma_start(tl[:], xlo[k * P:(k + 1) * P, :])
                los.append(tl)

            for n in range(NTI):
                pts = [pp.tile([P, NT], _F32, tag="p", name=f"p{n}_{m}")
                       for m in range(MT)]
                for k in range(KT):
                    wt = wp.tile([P, NT], _BF16, tag="w")
                    nc.sync.dma_start(
                        wt[:], w[k * P:(k + 1) * P, n * NT:(n + 1) * NT])
                    for m in range(MT):
                        nc.tensor.matmul(
                            pts[m][:], his[k][:, m * P:(m + 1) * P], wt[:],
                            start=(k == 0), stop=False)
                        nc.tensor.matmul(
                            pts[m][:], los[k][:, m * P:(m + 1) * P], wt[:],
                            start=False, stop=(k == KT - 1))
                for m in range(MT):
                    ot = op.tile([P, NT], _F32, tag="o")
                    nc.vector.tensor_copy(ot[:], pts[m][:])
                    nc.sync.dma_start(
                        out[m * P:(m + 1) * P, n * NT:(n + 1) * NT], ot[:])
    nc.compile()
    return nc


def _get_module():
    global _cached
    if _cached is None:
        _cached = _build()
    return _cached


def _run(inputs: np.ndarray, kernel_w: np.ndarray, trace: bool = False):
    nc = _get_module()

    bw = (kernel_w > 0.5).astype(ml_dtypes.bfloat16)
    hi = inputs.astype(ml_dtypes.bfloat16)
    lo = (inputs - hi.astype(np.float32)).astype(ml_dtypes.bfloat16)

    in_maps = []
    for i in range(N_CORES):
        sl = slice(i * TOK, (i + 1) * TOK)
        in_maps.append({
            "xhi": np.ascontiguousarray(hi[sl].T),
            "xlo": np.ascontiguousarray(lo[sl].T),
            "w": bw,
        })

    res = run_bass_kernel_spmd(nc, in_maps, core_ids=list(range(N_CORES)),
                               trace=trace)
    full = np.concatenate([r["out"] for r in res.results], axis=0)
    return full, res


def kernel(inputs: np.ndarray, kernel: np.ndarray) -> np.ndarray:
    return _run(inputs, kernel)[0]


# revision 3
# speedup vs baseline: 1.0463x; 1.0463x over previous
"""BinarizedDense TRN2 kernel: out = inputs @ (kernel > 0.5).

inputs [8192, 4096] f32, kernel [4096, 4096] f32 -> out [8192, 4096] f32.

Strategy
--------
Data parallel over tokens: each of the 8 NeuronCores computes a
[1024, 4096] output shard against the full binarized weight matrix.

Per core the matmul runs on the PE array in bf16 with an exact hi/lo
significand split of the activations: since the binarized weights are
exactly representable in bf16 ({0., 1.}), each product bf16(x)*b is
exact, so out = hi@B + lo@B accumulated in fp32 PSUM carries ~2^-17
relative representation error - on par with a plain fp32 matmul - while
streaming the PE at 1 cycle/row (4x the fp32 matmul rate).

Layout per core: activations are staged transposed ([in_f, tok], K on
partitions) and SBUF-resident (hi+lo = 128 KB/partition); the weight
matrix streams from HBM once as bf16 [128, 512] tiles; PSUM accumulates
over the 32 K-tiles; 8 token-tiles of one 512-wide output block occupy
the 8 PSUM banks; DVE evicts PSUM->SBUF and DMA writes the fp32 output.
"""
from contextlib import ExitStack

import numpy as np
import ml_dtypes

import concourse.bass as bass
import concourse.tile as tile
from concourse import bacc, mybir
from concourse.bass_utils import run_bass_kernel_spmd

TOKENS, IN_F, OUT_F = 8192, 4096, 4096
N_CORES = 8
TOK = TOKENS // N_CORES      # 1024 tokens per core
P = 128                      # partitions
NT = 512                     # output free-dim tile (one PSUM bank of fp32)
KT = IN_F // P               # 32 contraction tiles
MT = TOK // P                # 8 token tiles per core
NTI = OUT_F // NT            # 8 output blocks

_BF16 = mybir.dt.bfloat16
_F32 = mybir.dt.float32

_cached = None


def _build():
    nc = bacc.Bacc("TRN2", target_bir_lowering=False, debug=False)
    xhi = nc.dram_tensor("xhi", [IN_F, TOK], _BF16, kind="ExternalInput").ap()
    xlo = nc.dram_tensor("xlo", [IN_F, TOK], _BF16, kind="ExternalInput").ap()
    w = nc.dram_tensor("w", [IN_F, OUT_F], _BF16, kind="ExternalInput").ap()
    out = nc.dram_tensor("out", [TOK, OUT_F], _F32, kind="ExternalOutput").ap()

    with tile.TileContext(nc) as tc:
        with ExitStack() as ctx:
            xp = ctx.enter_context(tc.tile_pool(name="x", bufs=2 * KT))
            wp = ctx.enter_context(tc.tile_pool(name="w", bufs=8))
            op = ctx.enter_context(tc.tile_pool(name="o", bufs=8))
            pp = ctx.enter_context(tc.tile_pool(name="p", bufs=8, space="PSUM"))

            # Activation tiles load just-in-time during output block 0,
            # interleaved with the weight stream so the first matmul isn't
            # queued behind all 64 activation DMAs.
            his, los = [], []

            for n in range(NTI):
                pts = [pp.tile([P, NT], _F32, tag="p", name=f"p{n}_{m}")
                       for m in range(MT)]
                for k in range(KT):
                    if n == 0:
                        th = xp.tile([P, TOK], _BF16, tag="x", name=f"xh{k}")
                        nc.sync.dma_start(th[:], xhi[k * P:(k + 1) * P, :])
                        his.append(th)
                        tl = xp.tile([P, TOK], _BF16, tag="x", name=f"xl{k}")
                        nc.sync.dma_start(tl[:], xlo[k * P:(k + 1) * P, :])
                        los.append(tl)
                    wt = wp.tile([P, NT], _BF16, tag="w")
                    nc.sync.dma_start(
                        wt[:], w[k * P:(k + 1) * P, n * NT:(n + 1) * NT])
                    for m in range(MT):
                        nc.tensor.matmul(
                            pts[m][:], his[k][:, m * P:(m + 1) * P], wt[:],
                            start=(k == 0), stop=False)
                        nc.tensor.matmul(
                            pts[m][:], los[k][:, m * P:(m + 1) * P], wt[:],
                            start=False, stop=(k == KT - 1))
                for m in range(MT):
                    ot = op.tile([P, NT], _F32, tag="o")
                    nc.vector.tensor_copy(ot[:], pts[m][:])
                    nc.sync.dma_start(
                        out[m * P:(m + 1) * P, n * NT:(n + 1) * NT], ot[:])
    nc.compile()
    return nc


def _get_module():
    global _cached
    if _cached is None:
        _cached = _build()
    return _cached


def _run(inputs: np.ndarray, kernel_w: np.ndarray, trace: bool = False):
    nc = _get_module()

    bw = (kernel_w > 0.5).astype(ml_dtypes.bfloat16)
    hi = inputs.astype(ml_dtypes.bfloat16)
    lo = (inputs - hi.astype(np.float32)).astype(ml_dtypes.bfloat16)

    in_maps = []
    for i in range(N_CORES):
        sl = slice(i * TOK, (i + 1) * TOK)
        in_maps.append({
            "xhi": np.ascontiguousarray(hi[sl].T),
            "xlo": np.ascontiguousarray(lo[sl].T),
            "w": bw,
        })

    res = run_bass_kernel_spmd(nc, in_maps, core_ids=list(range(N_CORES)),
                               trace=trace)
    full = np.concatenate([r["out"] for r in res.results], axis=0)
    return full, res


def kernel(inputs: np.ndarray, kernel: np.ndarray) -> np.ndarray:
    return _run(inputs, kernel)[0]


# revision 6
# speedup vs baseline: 1.3545x; 1.2946x over previous
"""BinarizedDense TRN2 kernel: out = inputs @ (kernel > 0.5).

inputs [8192, 4096] f32, kernel [4096, 4096] f32 -> out [8192, 4096] f32.

Strategy
--------
Data parallel over tokens: each of the 8 NeuronCores computes a
[1024, 4096] output shard against the full binarized weight matrix.

Per core the matmul runs on the PE array with an exact two-term
significand split of the activations, exploiting that the binarized
weights are exactly representable at low precision:

  x = fp16(x) + lo,   lo8 = e4m3(lo * 2^9),   w8 = e4m3(B * 2^-9)
  out = fp16(x) @ B   (fp16 matmuls, 1 cycle/row)
      + lo8 @ w8      (fp8 DoubleRow matmuls, 0.5 cycle/row)

Both B and B*2^-9 are exact in their dtypes, so every product is exact
and the only approximation is rounding lo*2^9 to e4m3 (~2^-15 |x|) plus
fp32 PSUM accumulation - measured ~5e-6 of output scale, on par with a
plain fp32 matmul's reduction-order envelope. The lo-pass matmuls
accumulate into the same PSUM group as the hi pass (the 2^9 prescale of
w8 makes them directly summable), so there is no merge pass.

Layout per core: activations staged transposed ([in_f, tok], K on
partitions), SBUF-resident (96 KB/partition); weights stream from HBM
once per output block; PSUM accumulates over K; the 8 token-tiles of a
512-wide output block occupy the 8 PSUM banks; DVE evicts PSUM->SBUF
and DMA writes the fp32 output. Activation loads are emitted just in
time inside output block 0 so the first weight tile isn't queued behind
them.
"""
from contextlib import ExitStack

import numpy as np
import ml_dtypes

import concourse.bass as bass
import concourse.tile as tile
from concourse import bacc, mybir
from concourse.bass_utils import run_bass_kernel_spmd

TOKENS, IN_F, OUT_F = 8192, 4096, 4096
N_CORES = 8
TOK = TOKENS // N_CORES      # 1024 tokens per core
P = 128                      # partitions
NT = 512                     # output free-dim tile (one PSUM bank of fp32)
KT = IN_F // P               # 32 contraction tiles
KP = KT // 2                 # 16 contraction tile pairs (DoubleRow)
MT = TOK // P                # 8 token tiles per core
NTI = OUT_F // NT            # 8 output blocks

LO_SCALE = 512.0             # 2^9: B/512 is still exact in e4m3

_F16 = mybir.dt.float16
_F8 = mybir.dt.float8e4
_F32 = mybir.dt.float32

_cached = None


def _build():
    nc = bacc.Bacc("TRN2", target_bir_lowering=False, debug=False)
    xhi = nc.dram_tensor("xhi", [IN_F, TOK], _F16, kind="ExternalInput").ap()
    xlo = nc.dram_tensor("xlo", [IN_F, TOK], _F8, kind="ExternalInput").ap()
    whi = nc.dram_tensor("whi", [IN_F, OUT_F], _F16, kind="ExternalInput").ap()
    wlo = nc.dram_tensor("wlo", [IN_F, OUT_F], _F8, kind="ExternalInput").ap()
    out = nc.dram_tensor("out", [TOK, OUT_F], _F32, kind="ExternalOutput").ap()

    with tile.TileContext(nc) as tc:
        with ExitStack() as ctx:
            xp = ctx.enter_context(tc.tile_pool(name="x", bufs=1))
            wp = ctx.enter_context(tc.tile_pool(name="w", bufs=1))
            op = ctx.enter_context(tc.tile_pool(name="o", bufs=8))
            pp = ctx.enter_context(tc.tile_pool(name="p", bufs=8, space="PSUM"))

            his = []   # KT fp16 tiles [P, TOK]
            los = []   # KP fp8 pair tiles [P, 2, TOK]

            for n in range(NTI):
                pts = [pp.tile([P, NT], _F32, tag="p", name=f"p{n}_{m}")
                       for m in range(MT)]
                for k2 in range(KP):
                    if n == 0:
                        for j in range(2):
                            k = 2 * k2 + j
                            th = xp.tile([P, TOK], _F16, tag="xh", name=f"xh{k}", bufs=KT)
                            nc.sync.dma_start(th[:], xhi[k * P:(k + 1) * P, :])
                            his.append(th)
                        tl = xp.tile([P, 2, TOK], _F8, tag="xl", name=f"xl{k2}", bufs=KP)
                        nc.sync.dma_start(
                            tl[:],
                            xlo[k2 * 2 * P:(k2 + 1) * 2 * P, :].rearrange(
                                "(j p) m -> p j m", p=P))
                        los.append(tl)
                    wha = wp.tile([P, NT], _F16, tag="wh", name=f"wha{n}_{k2}", bufs=10)
                    nc.sync.dma_start(
                        wha[:], whi[2 * k2 * P:(2 * k2 + 1) * P,
                                    n * NT:(n + 1) * NT])
                    whb = wp.tile([P, NT], _F16, tag="wh", name=f"whb{n}_{k2}", bufs=10)
                    nc.sync.dma_start(
                        whb[:], whi[(2 * k2 + 1) * P:(2 * k2 + 2) * P,
                                    n * NT:(n + 1) * NT])
                    wl = wp.tile([P, 2, NT], _F8, tag="wl", name=f"wl{n}_{k2}", bufs=5)
                    nc.sync.dma_start(
                        wl[:],
                        wlo[k2 * 2 * P:(k2 + 1) * 2 * P,
                            n * NT:(n + 1) * NT].rearrange(
                            "(j p) m -> p j m", p=P))
                    for m in range(MT):
                        ms = slice(m * P, (m + 1) * P)
                        nc.tensor.matmul(
                            pts[m][:], his[2 * k2][:, ms], wha[:],
                            start=(k2 == 0), stop=False)
                        nc.tensor.matmul(
                            pts[m][:], his[2 * k2 + 1][:, ms], whb[:],
                            start=False, stop=False)
                        nc.tensor.matmul(
                            pts[m][:], los[k2][:, :, ms], wl[:],
                            start=False, stop=(k2 == KP - 1),
                            perf_mode=mybir.MatmulPerfMode.DoubleRow)
                for m in range(MT):
                    ot = op.tile([P, NT], _F32, tag="o", name=f"o{n}_{m}")
                    nc.vector.tensor_copy(ot[:], pts[m][:])
                    nc.sync.dma_start(
                        out[m * P:(m + 1) * P, n * NT:(n + 1) * NT], ot[:])
    nc.compile()
    return nc


def _get_module():
    global _cached
    if _cached is None:
        _cached = _build()
    return _cached


def _run(inputs: np.ndarray, kernel_w: np.ndarray, trace: bool = False):
    nc = _get_module()

    bw = kernel_w > 0.5
    whi = bw.astype(np.float16)
    wlo = (bw.astype(np.float32) / LO_SCALE).astype(ml_dtypes.float8_e4m3)
    hi = inputs.astype(np.float16)
    lo = ((inputs - hi.astype(np.float32)) * LO_SCALE).astype(
        ml_dtypes.float8_e4m3)

    in_maps = []
    for i in range(N_CORES):
        sl = slice(i * TOK, (i + 1) * TOK)
        in_maps.append({
            "xhi": np.ascontiguousarray(hi[sl].T),
            "xlo": np.ascontiguousarray(lo[sl].T),
            "whi": whi,
            "wlo": wlo,
        })

    res = run_bass_kernel_spmd(nc, in_maps, core_ids=list(range(N_CORES)),
                               trace=trace)
    full = np.concatenate([r["out"] for r in res.results], axis=0)
    return full, res


def kernel(inputs: np.ndarray, kernel: np.ndarray) -> np.ndarray:
    return _run(inputs, kernel)[0]


# revision 8
# speedup vs baseline: 1.3548x; 1.0002x over previous
"""BinarizedDense TRN2 kernel: out = inputs @ (kernel > 0.5).

inputs [8192, 4096] f32, kernel [4096, 4096] f32 -> out [8192, 4096] f32.

Strategy
--------
Data parallel over tokens: each of the 8 NeuronCores computes a
[1024, 4096] output shard against the full binarized weight matrix.

Per core the matmul runs on the PE array with an exact two-term
significand split of the activations, exploiting that the binarized
weights are exactly representable at low precision:

  x = fp16(x) + lo,   lo8 = e4m3(lo * 2^9),   w8 = e4m3(B * 2^-9)
  out = fp16(x) @ B   (fp16 matmuls, 1 cycle/row)
      + lo8 @ w8      (fp8 DoubleRow matmuls, 0.5 cycle/row)

Both B and B*2^-9 are exact in their dtypes, so every product is exact
and the only approximation is rounding lo*2^9 to e4m3 (~2^-15 |x|) plus
fp32 PSUM accumulation - measured ~5e-6 of output scale, on par with a
plain fp32 matmul's reduction-order envelope. The lo-pass matmuls
accumulate into the same PSUM group as the hi pass (the 2^9 prescale of
w8 makes them directly summable), so there is no merge pass.

Layout per core: activations staged transposed ([in_f, tok], K on
partitions), SBUF-resident (96 KB/partition); weights stream from HBM
once per output block; PSUM accumulates over K; the 8 token-tiles of a
512-wide output block occupy the 8 PSUM banks; DVE evicts PSUM->SBUF
and DMA writes the fp32 output. Activation loads are emitted just in
time inside output block 0 so the first weight tile isn't queued behind
them.
"""
from contextlib import ExitStack

import numpy as np
import ml_dtypes

import concourse.bass as bass
import concourse.tile as tile
from concourse import bacc, mybir
from concourse.bass_utils import run_bass_kernel_spmd

TOKENS, IN_F, OUT_F = 8192, 4096, 4096
N_CORES = 8
TOK = TOKENS // N_CORES      # 1024 tokens per core
P = 128                      # partitions
NT = 512                     # output free-dim tile (one PSUM bank of fp32)
KT = IN_F // P               # 32 contraction tiles
KP = KT // 2                 # 16 contraction tile pairs (DoubleRow)
MT = TOK // P                # 8 token tiles per core
NTI = OUT_F // NT            # 8 output blocks

LO_SCALE = 512.0             # 2^9: B/512 is still exact in e4m3

_F16 = mybir.dt.float16
_F8 = mybir.dt.float8e4
_F32 = mybir.dt.float32

_cached = None


def _build():
    nc = bacc.Bacc("TRN2", target_bir_lowering=False, debug=False)
    xhi = nc.dram_tensor("xhi", [IN_F, TOK], _F16, kind="ExternalInput").ap()
    xlo = nc.dram_tensor("xlo", [IN_F, TOK], _F8, kind="ExternalInput").ap()
    whi = nc.dram_tensor("whi", [IN_F, OUT_F], _F16, kind="ExternalInput").ap()
    wlo = nc.dram_tensor("wlo", [IN_F, OUT_F], _F8, kind="ExternalInput").ap()
    out = nc.dram_tensor("out", [TOK, OUT_F], _F32, kind="ExternalOutput").ap()

    with tile.TileContext(nc) as tc:
        with ExitStack() as ctx:
            xp = ctx.enter_context(tc.tile_pool(name="x", bufs=1))
            wp = ctx.enter_context(tc.tile_pool(name="w", bufs=1))
            op = ctx.enter_context(tc.tile_pool(name="o", bufs=8))
            pp = ctx.enter_context(tc.tile_pool(name="p", bufs=8, space="PSUM"))

            his = []   # KT fp16 tiles [P, TOK]
            los = []   # KP fp8 pair tiles [P, 2, TOK]

            # Warm the PE clock (HAM releases the 1.2 GHz throttle after
            # ~3.4 us of sustained activity) during the initial DMA wait,
            # so the first real matmuls run at 2.4 GHz.
            warm = wp.tile([P, NT], _F16, tag="warm", name="warm", bufs=1)
            nc.any.memset(warm[:], 0.0)
            pwarm = pp.tile([P, NT], _F32, tag="p", name="pwarm", bufs=8)
            for i in range(24):
                nc.tensor.matmul(pwarm[:], warm[:, :P], warm[:],
                                 start=True, stop=True)

            for n in range(NTI):
                pts = [pp.tile([P, NT], _F32, tag="p", name=f"p{n}_{m}")
                       for m in range(MT)]
                for k2 in range(KP):
                    # Interleave weight and (block-0 only) activation loads
                    # so the first matmul's operands land earliest.
                    wha = wp.tile([P, NT], _F16, tag="wh", name=f"wha{n}_{k2}", bufs=10)
                    nc.sync.dma_start(
                        wha[:], whi[2 * k2 * P:(2 * k2 + 1) * P,
                                    n * NT:(n + 1) * NT])
                    if n == 0:
                        th = xp.tile([P, TOK], _F16, tag="xh",
                                     name=f"xh{2 * k2}", bufs=KT)
                        nc.sync.dma_start(
                            th[:], xhi[2 * k2 * P:(2 * k2 + 1) * P, :])
                        his.append(th)
                    whb = wp.tile([P, NT], _F16, tag="wh", name=f"whb{n}_{k2}", bufs=10)
                    nc.sync.dma_start(
                        whb[:], whi[(2 * k2 + 1) * P:(2 * k2 + 2) * P,
                                    n * NT:(n + 1) * NT])
                    if n == 0:
                        th = xp.tile([P, TOK], _F16, tag="xh",
                                     name=f"xh{2 * k2 + 1}", bufs=KT)
                        nc.sync.dma_start(
                            th[:], xhi[(2 * k2 + 1) * P:(2 * k2 + 2) * P, :])
                        his.append(th)
                    wl = wp.tile([P, 2, NT], _F8, tag="wl", name=f"wl{n}_{k2}", bufs=5)
                    nc.sync.dma_start(
                        wl[:],
                        wlo[k2 * 2 * P:(k2 + 1) * 2 * P,
                            n * NT:(n + 1) * NT].rearrange(
                            "(j p) m -> p j m", p=P))
                    if n == 0:
                        tl = xp.tile([P, 2, TOK], _F8, tag="xl",
                                     name=f"xl{k2}", bufs=KP)
                        nc.sync.dma_start(
                            tl[:],
                            xlo[k2 * 2 * P:(k2 + 1) * 2 * P, :].rearrange(
                                "(j p) m -> p j m", p=P))
                        los.append(tl)
                    for m in range(MT):
                        ms = slice(m * P, (m + 1) * P)
                        nc.tensor.matmul(
                            pts[m][:], his[2 * k2][:, ms], wha[:],
                            start=(k2 == 0), stop=False)
                        nc.tensor.matmul(
                            pts[m][:], his[2 * k2 + 1][:, ms], whb[:],
                            start=False, stop=False)
                        nc.tensor.matmul(
                            pts[m][:], los[k2][:, :, ms], wl[:],
                            start=False, stop=(k2 == KP - 1),
                            perf_mode=mybir.MatmulPerfMode.DoubleRow)
                for m in range(MT):
                    ot = op.tile([P, NT], _F32, tag="o", name=f"o{n}_{m}")
                    nc.vector.tensor_copy(ot[:], pts[m][:])
                    nc.sync.dma_start(
                        out[m * P:(m + 1) * P, n * NT:(n + 1) * NT], ot[:])
    nc.compile()
    return nc


def _get_module():
    global _cached
    if _cached is None:
        _cached = _build()
    return _cached


def _run(inputs: np.ndarray, kernel_w: np.ndarray, trace: bool = False):
    nc = _get_module()

    bw = kernel_w > 0.5
    whi = bw.astype(np.float16)
    wlo = (bw.astype(np.float32) / LO_SCALE).astype(ml_dtypes.float8_e4m3)
    hi = inputs.astype(np.float16)
    lo = ((inputs - hi.astype(np.float32)) * LO_SCALE).astype(
        ml_dtypes.float8_e4m3)

    in_maps = []
    for i in range(N_CORES):
        sl = slice(i * TOK, (i + 1) * TOK)
        in_maps.append({
            "xhi": np.ascontiguousarray(hi[sl].T),
            "xlo": np.ascontiguousarray(lo[sl].T),
            "whi": whi,
            "wlo": wlo,
        })

    res = run_bass_kernel_spmd(nc, in_maps, core_ids=list(range(N_CORES)),
                               trace=trace)
    full = np.concatenate([r["out"] for r in res.results], axis=0)
    return full, res


def kernel(inputs: np.ndarray, kernel: np.ndarray) -> np.ndarray:
    return _run(inputs, kernel)[0]


# revision 10
# speedup vs baseline: 1.3560x; 1.0009x over previous
"""BinarizedDense TRN2 kernel: out = inputs @ (kernel > 0.5).

inputs [8192, 4096] f32, kernel [4096, 4096] f32 -> out [8192, 4096] f32.

Strategy
--------
Data parallel over tokens: each of the 8 NeuronCores computes a
[1024, 4096] output shard against the full binarized weight matrix.

Per core the matmul runs on the PE array with an exact two-term
significand split of the activations, exploiting that the binarized
weights are exactly representable at low precision:

  x = fp16(x) + lo,   lo8 = e4m3(lo * 2^9),   w8 = e4m3(B * 2^-9)
  out = fp16(x) @ B   (fp16 matmuls, 1 cycle/row)
      + lo8 @ w8      (fp8 DoubleRow matmuls, 0.5 cycle/row)

Both B and B*2^-9 are exact in their dtypes, so every product is exact
and the only approximation is rounding lo*2^9 to e4m3 (~2^-15 |x|) plus
fp32 PSUM accumulation - measured ~5e-6 of output scale, on par with a
plain fp32 matmul's reduction-order envelope. The lo-pass matmuls
accumulate into the same PSUM group as the hi pass (the 2^9 prescale of
w8 makes them directly summable), so there is no merge pass.

Layout per core: activations staged transposed ([in_f, tok], K on
partitions), SBUF-resident (96 KB/partition); weights stream from HBM
once per output block; PSUM accumulates over K; the 8 token-tiles of a
512-wide output block occupy the 8 PSUM banks; DVE evicts PSUM->SBUF
and DMA writes the fp32 output. Activation loads are emitted just in
time inside output block 0 so the first weight tile isn't queued behind
them.
"""
from contextlib import ExitStack

import numpy as np
import ml_dtypes

import concourse.bass as bass
import concourse.tile as tile
from concourse import bacc, mybir
from concourse.bass_utils import run_bass_kernel_spmd

TOKENS, IN_F, OUT_F = 8192, 4096, 4096
N_CORES = 8
TOK = TOKENS // N_CORES      # 1024 tokens per core
P = 128                      # partitions
NT = 512                     # output free-dim tile (one PSUM bank of fp32)
KT = IN_F // P               # 32 contraction tiles
KP = KT // 2                 # 16 contraction tile pairs (DoubleRow)
MT = TOK // P                # 8 token tiles per core
NTI = OUT_F // NT            # 8 output blocks

LO_SCALE = 512.0             # 2^9: B/512 is still exact in e4m3

_F16 = mybir.dt.float16
_F8 = mybir.dt.float8e4
_F32 = mybir.dt.float32

_cached = None


def _build():
    nc = bacc.Bacc("TRN2", target_bir_lowering=False, debug=False)
    xhi = nc.dram_tensor("xhi", [IN_F, TOK], _F16, kind="ExternalInput").ap()
    xlo = nc.dram_tensor("xlo", [IN_F, TOK], _F8, kind="ExternalInput").ap()
    whi = nc.dram_tensor("whi", [IN_F, OUT_F], _F16, kind="ExternalInput").ap()
    wlo = nc.dram_tensor("wlo", [IN_F, OUT_F], _F8, kind="ExternalInput").ap()
    out = nc.dram_tensor("out", [TOK, OUT_F], _F32, kind="ExternalOutput").ap()

    with tile.TileContext(nc) as tc:
        with ExitStack() as ctx:
            xp = ctx.enter_context(tc.tile_pool(name="x", bufs=1))
            wp = ctx.enter_context(tc.tile_pool(name="w", bufs=1))
            op = ctx.enter_context(tc.tile_pool(name="o", bufs=8))
            pp = ctx.enter_context(tc.tile_pool(name="p", bufs=8, space="PSUM"))

            his = []   # KT fp16 tiles [P, TOK]
            los = []   # KP fp8 pair tiles [P, 2, TOK]

            # Warm the PE clock (HAM releases the 1.2 GHz throttle after
            # ~3.4 us of sustained activity) during the initial DMA wait,
            # so the first real matmuls run at 2.4 GHz.
            warm = wp.tile([P, NT], _F16, tag="warm", name="warm", bufs=1)
            nc.any.memset(warm[:], 0.0)
            pwarm = pp.tile([P, NT], _F32, tag="p", name="pwarm", bufs=8)
            for i in range(24):
                nc.tensor.matmul(pwarm[:], warm[:, :P], warm[:],
                                 start=True, stop=True)

            for n in range(NTI):
                pts = [pp.tile([P, NT], _F32, tag="p", name=f"p{n}_{m}")
                       for m in range(MT)]
                for k2 in range(KP):
                    # Interleave weight and (block-0 only) activation loads
                    # so the first matmul's operands land earliest.
                    wha = wp.tile([P, NT], _F16, tag="wh", name=f"wha{n}_{k2}", bufs=10)
                    nc.sync.dma_start(
                        wha[:], whi[2 * k2 * P:(2 * k2 + 1) * P,
                                    n * NT:(n + 1) * NT])
                    if n == 0:
                        th = xp.tile([P, TOK], _F16, tag="xh",
                                     name=f"xh{2 * k2}", bufs=KT)
                        nc.sync.dma_start(
                            th[:], xhi[2 * k2 * P:(2 * k2 + 1) * P, :])
                        his.append(th)
                    whb = wp.tile([P, NT], _F16, tag="wh", name=f"whb{n}_{k2}", bufs=10)
                    nc.sync.dma_start(
                        whb[:], whi[(2 * k2 + 1) * P:(2 * k2 + 2) * P,
                                    n * NT:(n + 1) * NT])
                    if n == 0:
                        th = xp.tile([P, TOK], _F16, tag="xh",
                                     name=f"xh{2 * k2 + 1}", bufs=KT)
                        nc.sync.dma_start(
                            th[:], xhi[(2 * k2 + 1) * P:(2 * k2 + 2) * P, :])
                        his.append(th)
                    wl = wp.tile([P, 2, NT], _F8, tag="wl", name=f"wl{n}_{k2}", bufs=5)
                    nc.sync.dma_start(
                        wl[:],
                        wlo[k2 * 2 * P:(k2 + 1) * 2 * P,
                            n * NT:(n + 1) * NT].rearrange(
                            "(j p) m -> p j m", p=P))
                    if n == 0:
                        tl = xp.tile([P, 2, TOK], _F8, tag="xl",
                                     name=f"xl{k2}", bufs=KP)
                        nc.sync.dma_start(
                            tl[:],
                            xlo[k2 * 2 * P:(k2 + 1) * 2 * P, :].rearrange(
                                "(j p) m -> p j m", p=P))
                        los.append(tl)
                    for m in range(MT):
                        ms = slice(m * P, (m + 1) * P)
                        nc.tensor.matmul(
                            pts[m][:], his[2 * k2][:, ms], wha[:],
                            start=(k2 == 0), stop=False)
                        nc.tensor.matmul(
                            pts[m][:], his[2 * k2 + 1][:, ms], whb[:],
                            start=False, stop=False)
                        nc.tensor.matmul(
                            pts[m][:], los[k2][:, :, ms], wl[:],
                            start=False, stop=(k2 == KP - 1),
                            perf_mode=mybir.MatmulPerfMode.DoubleRow)
                for m in range(MT):
                    ot = op.tile([P, NT], _F32, tag="o", name=f"o{n}_{m}")
                    nc.vector.tensor_copy(ot[:], pts[m][:])
                    nc.sync.dma_start(
                        out[m * P:(m + 1) * P, n * NT:(n + 1) * NT], ot[:])
    nc.compile()
    return nc


def _get_module():
    global _cached
    if _cached is None:
        _cached = _build()
    return _cached


def _run(inputs: np.ndarray, kernel_w: np.ndarray, trace: bool = False):
    nc = _get_module()

    bw = kernel_w > 0.5
    whi = bw.astype(np.float16)
    wlo = (bw.astype(np.float32) / LO_SCALE).astype(ml_dtypes.float8_e4m3)
    hi = inputs.astype(np.float16)
    lo = ((inputs - hi.astype(np.float32)) * LO_SCALE).astype(
        ml_dtypes.float8_e4m3)

    in_maps = []
    for i in range(N_CORES):
        sl = slice(i * TOK, (i + 1) * TOK)
        in_maps.append({
            "xhi": np.ascontiguousarray(hi[sl].T),
            "xlo": np.ascontiguousarray(lo[sl].T),
            "whi": whi,
            "wlo": wlo,
        })

    res = run_bass_kernel_spmd(nc, in_maps, core_ids=list(range(N_CORES)),
                               trace=trace)
    full = np.concatenate([r["out"] for r in res.results], axis=0)
    return full, res


def kernel(inputs: np.ndarray, kernel: np.ndarray) -> np.ndarray:
    return _run(inputs, kernel)[0]


# revision 11
# speedup vs baseline: 1.3730x; 1.0125x over previous
"""BinarizedDense TRN2 kernel: out = inputs @ (kernel > 0.5).

inputs [8192, 4096] f32, kernel [4096, 4096] f32 -> out [8192, 4096] f32.

Strategy
--------
Data parallel over tokens: each of the 8 NeuronCores computes a
[1024, 4096] output shard against the full binarized weight matrix.

Per core the matmul runs on the PE array with an exact two-term
significand split of the activations, exploiting that the binarized
weights are exactly representable at low precision:

  x = fp16(x) + lo,   lo8 = e4m3(lo * 2^9),   w8 = e4m3(B * 2^-9)
  out = fp16(x) @ B   (fp16 matmuls, 1 cycle/row)
      + lo8 @ w8      (fp8 DoubleRow matmuls, 0.5 cycle/row)

Both B and B*2^-9 are exact in their dtypes, so every product is exact
and the only approximation is rounding lo*2^9 to e4m3 (~2^-15 |x|) plus
fp32 PSUM accumulation - measured ~5e-6 of output scale, on par with a
plain fp32 matmul's reduction-order envelope. The lo-pass matmuls
accumulate into the same PSUM group as the hi pass (the 2^9 prescale of
w8 makes them directly summable), so there is no merge pass.

Layout per core: activations staged transposed ([in_f, tok], K on
partitions), SBUF-resident (96 KB/partition); weights stream from HBM
once per output block; PSUM accumulates over K; the 8 token-tiles of a
512-wide output block occupy the 8 PSUM banks; DVE evicts PSUM->SBUF
and DMA writes the fp32 output. Activation loads are emitted just in
time inside output block 0 so the first weight tile isn't queued behind
them.
"""
from contextlib import ExitStack

import numpy as np
import ml_dtypes

import concourse.bass as bass
import concourse.tile as tile
from concourse import bacc, mybir
from concourse.bass_utils import run_bass_kernel_spmd

TOKENS, IN_F, OUT_F = 8192, 4096, 4096
N_CORES = 8
TOK = TOKENS // N_CORES      # 1024 tokens per core
P = 128                      # partitions
NT = 512                     # output free-dim tile (one PSUM bank of fp32)
KT = IN_F // P               # 32 contraction tiles
KP = KT // 2                 # 16 contraction tile pairs (DoubleRow)
MT = TOK // P                # 8 token tiles per core
NTI = OUT_F // NT            # 8 output blocks

LO_SCALE = 512.0             # 2^9: B/512 is still exact in e4m3

_F16 = mybir.dt.float16
_F8 = mybir.dt.float8e4
_F32 = mybir.dt.float32

_cached = None


def _build():
    nc = bacc.Bacc("TRN2", target_bir_lowering=False, debug=False)
    xhi = nc.dram_tensor("xhi", [IN_F, TOK], _F16, kind="ExternalInput").ap()
    xlo = nc.dram_tensor("xlo", [IN_F, TOK], _F8, kind="ExternalInput").ap()
    whi = nc.dram_tensor("whi", [IN_F, OUT_F], _F16, kind="ExternalInput").ap()
    wlo = nc.dram_tensor("wlo", [IN_F, OUT_F], _F8, kind="ExternalInput").ap()
    out = nc.dram_tensor("out", [TOK, OUT_F], _F32, kind="ExternalOutput").ap()

    with tile.TileContext(nc) as tc:
        with ExitStack() as ctx:
            xp = ctx.enter_context(tc.tile_pool(name="x", bufs=1))
            wp = ctx.enter_context(tc.tile_pool(name="w", bufs=1))
            op = ctx.enter_context(tc.tile_pool(name="o", bufs=8))
            pp = ctx.enter_context(tc.tile_pool(name="p", bufs=8, space="PSUM"))

            his = []   # KT fp16 tiles [P, TOK]
            los = []   # KP fp8 pair tiles [P, 2, TOK]

            # Warm the PE clock (HAM releases the 1.2 GHz throttle after
            # ~3.4 us of sustained activity) during the initial DMA wait,
            # so the first real matmuls run at 2.4 GHz.
            warm = wp.tile([P, NT], _F16, tag="warm", name="warm", bufs=1)
            nc.any.memset(warm[:], 0.0)
            pwarm = pp.tile([P, NT], _F32, tag="p", name="pwarm", bufs=8)
            for i in range(24):
                nc.tensor.matmul(pwarm[:], warm[:, :P], warm[:],
                                 start=True, stop=True)

            for n in range(NTI):
                pts = [pp.tile([P, NT], _F32, tag="p", name=f"p{n}_{m}")
                       for m in range(MT)]
                for k2 in range(KP):
                    # Interleave weight and (block-0 only) activation loads
                    # so the first matmul's operands land earliest.
                    wha = wp.tile([P, NT], _F16, tag="wh", name=f"wha{n}_{k2}", bufs=10)
                    nc.sync.dma_start(
                        wha[:], whi[2 * k2 * P:(2 * k2 + 1) * P,
                                    n * NT:(n + 1) * NT])
                    if n == 0:
                        th = xp.tile([P, TOK], _F16, tag="xh",
                                     name=f"xh{2 * k2}", bufs=KT)
                        nc.sync.dma_start(
                            th[:], xhi[2 * k2 * P:(2 * k2 + 1) * P, :])
                        his.append(th)
                    whb = wp.tile([P, NT], _F16, tag="wh", name=f"whb{n}_{k2}", bufs=10)
                    nc.sync.dma_start(
                        whb[:], whi[(2 * k2 + 1) * P:(2 * k2 + 2) * P,
                                    n * NT:(n + 1) * NT])
                    if n == 0:
                        th = xp.tile([P, TOK], _F16, tag="xh",
                                     name=f"xh{2 * k2 + 1}", bufs=KT)
                        nc.sync.dma_start(
                            th[:], xhi[(2 * k2 + 1) * P:(2 * k2 + 2) * P, :])
                        his.append(th)
                    wl = wp.tile([P, 2, NT], _F8, tag="wl", name=f"wl{n}_{k2}", bufs=5)
                    nc.sync.dma_start(
                        wl[:],
                        wlo[k2 * 2 * P:(k2 + 1) * 2 * P,
                            n * NT:(n + 1) * NT].rearrange(
                            "(j p) m -> p j m", p=P))
                    if n == 0:
                        tl = xp.tile([P, 2, TOK], _F8, tag="xl",
                                     name=f"xl{k2}", bufs=KP)
                        nc.sync.dma_start(
                            tl[:],
                            xlo[k2 * 2 * P:(k2 + 1) * 2 * P, :].rearrange(
                                "(j p) m -> p j m", p=P))
                        los.append(tl)
                    # Batch same-mode matmuls to avoid PE mode thrash
                    # between plain and DoubleRow instructions.
                    for m in range(MT):
                        ms = slice(m * P, (m + 1) * P)
                        nc.tensor.matmul(
                            pts[m][:], his[2 * k2][:, ms], wha[:],
                            start=(k2 == 0), stop=False)
                        nc.tensor.matmul(
                            pts[m][:], his[2 * k2 + 1][:, ms], whb[:],
                            start=False, stop=False)
                    for m in range(MT):
                        ms = slice(m * P, (m + 1) * P)
                        nc.tensor.matmul(
                            pts[m][:], los[k2][:, :, ms], wl[:],
                            start=False, stop=(k2 == KP - 1),
                            perf_mode=mybir.MatmulPerfMode.DoubleRow)
                for m in range(MT):
                    ot = op.tile([P, NT], _F32, tag="o", name=f"o{n}_{m}")
                    nc.vector.tensor_copy(ot[:], pts[m][:])
                    nc.sync.dma_start(
                        out[m * P:(m + 1) * P, n * NT:(n + 1) * NT], ot[:])
    nc.compile()
    return nc


def _get_module():
    global _cached
    if _cached is None:
        _cached = _build()
    return _cached


def _run(inputs: np.ndarray, kernel_w: np.ndarray, trace: bool = False):
    nc = _get_module()

    bw = kernel_w > 0.5
    whi = bw.astype(np.float16)
    wlo = (bw.astype(np.float32) / LO_SCALE).astype(ml_dtypes.float8_e4m3)
    hi = inputs.astype(np.float16)
    lo = ((inputs - hi.astype(np.float32)) * LO_SCALE).astype(
        ml_dtypes.float8_e4m3)

    in_maps = []
    for i in range(N_CORES):
        sl = slice(i * TOK, (i + 1) * TOK)
        in_maps.append({
            "xhi": np.ascontiguousarray(hi[sl].T),
            "xlo": np.ascontiguousarray(lo[sl].T),
            "whi": whi,
            "wlo": wlo,
        })

    res = run_bass_kernel_spmd(nc, in_maps, core_ids=list(range(N_CORES)),
                               trace=trace)
    full = np.concatenate([r["out"] for r in res.results], axis=0)
    return full, res


def kernel(inputs: np.ndarray, kernel: np.ndarray) -> np.ndarray:
    return _run(inputs, kernel)[0]


# revision 12
# speedup vs baseline: 1.3809x; 1.0058x over previous
"""BinarizedDense TRN2 kernel: out = inputs @ (kernel > 0.5).

inputs [8192, 4096] f32, kernel [4096, 4096] f32 -> out [8192, 4096] f32.

Strategy
--------
Data parallel over tokens: each of the 8 NeuronCores computes a
[1024, 4096] output shard against the full binarized weight matrix.

Per core the matmul runs on the PE array with an exact two-term
significand split of the activations, exploiting that the binarized
weights are exactly representable at low precision:

  x = fp16(x) + lo,   lo8 = e4m3(lo * 2^9),   w8 = e4m3(B * 2^-9)
  out = fp16(x) @ B   (fp16 matmuls, 1 cycle/row)
      + lo8 @ w8      (fp8 DoubleRow matmuls, 0.5 cycle/row)

Both B and B*2^-9 are exact in their dtypes, so every product is exact
and the only approximation is rounding lo*2^9 to e4m3 (~2^-15 |x|) plus
fp32 PSUM accumulation - measured ~5e-6 of output scale, on par with a
plain fp32 matmul's reduction-order envelope. The lo-pass matmuls
accumulate into the same PSUM group as the hi pass (the 2^9 prescale of
w8 makes them directly summable), so there is no merge pass.

Layout per core: activations staged transposed ([in_f, tok], K on
partitions), SBUF-resident (96 KB/partition); weights stream from HBM
once per output block; PSUM accumulates over K; the 8 token-tiles of a
512-wide output block occupy the 8 PSUM banks; DVE evicts PSUM->SBUF
and DMA writes the fp32 output. Activation loads are emitted just in
time inside output block 0 so the first weight tile isn't queued behind
them.
"""
from contextlib import ExitStack

import numpy as np
import ml_dtypes

import concourse.bass as bass
import concourse.tile as tile
from concourse import bacc, mybir
from concourse.bass_utils import run_bass_kernel_spmd

TOKENS, IN_F, OUT_F = 8192, 4096, 4096
N_CORES = 8
TOK = TOKENS // N_CORES      # 1024 tokens per core
P = 128                      # partitions
NT = 512                     # output free-dim tile (one PSUM bank of fp32)
KT = IN_F // P               # 32 contraction tiles
KP = KT // 2                 # 16 contraction tile pairs (DoubleRow)
MT = TOK // P                # 8 token tiles per core
NTI = OUT_F // NT            # 8 output blocks

LO_SCALE = 512.0             # 2^9: B/512 is still exact in e4m3

_F16 = mybir.dt.float16
_F8 = mybir.dt.float8e4
_F32 = mybir.dt.float32

_cached = None


def _build():
    nc = bacc.Bacc("TRN2", target_bir_lowering=False, debug=False)
    xhi = nc.dram_tensor("xhi", [IN_F, TOK], _F16, kind="ExternalInput").ap()
    xlo = nc.dram_tensor("xlo", [IN_F, TOK], _F8, kind="ExternalInput").ap()
    whi = nc.dram_tensor("whi", [IN_F, OUT_F], _F16, kind="ExternalInput").ap()
    wlo = nc.dram_tensor("wlo", [IN_F, OUT_F], _F8, kind="ExternalInput").ap()
    out = nc.dram_tensor("out", [TOK, OUT_F], _F32, kind="ExternalOutput").ap()

    with tile.TileContext(nc) as tc:
        with ExitStack() as ctx:
            xp = ctx.enter_context(tc.tile_pool(name="x", bufs=1))
            wp = ctx.enter_context(tc.tile_pool(name="w", bufs=1))
            op = ctx.enter_context(tc.tile_pool(name="o", bufs=8))
            pp = ctx.enter_context(tc.tile_pool(name="p", bufs=8, space="PSUM"))

            his = []   # KT fp16 tiles [P, TOK]
            los = []   # KP fp8 pair tiles [P, 2, TOK]

            # Warm the PE clock (HAM releases the 1.2 GHz throttle after
            # ~3.4 us of sustained activity) during the initial DMA wait,
            # so the first real matmuls run at 2.4 GHz.
            warm = wp.tile([P, NT], _F16, tag="warm", name="warm", bufs=1)
            nc.any.memset(warm[:], 0.0)
            pwarm = pp.tile([P, NT], _F32, tag="p", name="pwarm", bufs=8)
            for i in range(24):
                nc.tensor.matmul(pwarm[:], warm[:, :P], warm[:],
                                 start=True, stop=True)

            for n in range(NTI):
                pts = [pp.tile([P, NT], _F32, tag="p", name=f"p{n}_{m}")
                       for m in range(MT)]
                for k2 in range(KP):
                    # Interleave weight and (block-0 only) activation loads
                    # so the first matmul's operands land earliest.
                    wha = wp.tile([P, NT], _F16, tag="wh", name=f"wha{n}_{k2}", bufs=10)
                    nc.sync.dma_start(
                        wha[:], whi[2 * k2 * P:(2 * k2 + 1) * P,
                                    n * NT:(n + 1) * NT])
                    if n == 0:
                        th = xp.tile([P, TOK], _F16, tag="xh",
                                     name=f"xh{2 * k2}", bufs=KT)
                        nc.sync.dma_start(
                            th[:], xhi[2 * k2 * P:(2 * k2 + 1) * P, :])
                        his.append(th)
                    whb = wp.tile([P, NT], _F16, tag="wh", name=f"whb{n}_{k2}", bufs=10)
                    nc.sync.dma_start(
                        whb[:], whi[(2 * k2 + 1) * P:(2 * k2 + 2) * P,
                                    n * NT:(n + 1) * NT])
                    if n == 0:
                        th = xp.tile([P, TOK], _F16, tag="xh",
                                     name=f"xh{2 * k2 + 1}", bufs=KT)
                        nc.sync.dma_start(
                            th[:], xhi[(2 * k2 + 1) * P:(2 * k2 + 2) * P, :])
                        his.append(th)
                    wl = wp.tile([P, 2, NT], _F8, tag="wl", name=f"wl{n}_{k2}", bufs=5)
                    nc.sync.dma_start(
                        wl[:],
                        wlo[k2 * 2 * P:(k2 + 1) * 2 * P,
                            n * NT:(n + 1) * NT].rearrange(
                            "(j p) m -> p j m", p=P))
                    if n == 0:
                        tl = xp.tile([P, 2, TOK], _F8, tag="xl",
                                     name=f"xl{k2}", bufs=KP)
                        nc.sync.dma_start(
                            tl[:],
                            xlo[k2 * 2 * P:(k2 + 1) * 2 * P, :].rearrange(
                                "(j p) m -> p j m", p=P))
                        los.append(tl)
                    # Batch same-mode matmuls to avoid PE mode thrash
                    # between plain and DoubleRow instructions - except on
                    # the last contraction step, where finishing bank m
                    # completely before bank m+1 lets PSUM evictions (and
                    # the next block's matmuls) start as early as possible.
                    last = k2 == KP - 1
                    for m in range(MT):
                        ms = slice(m * P, (m + 1) * P)
                        nc.tensor.matmul(
                            pts[m][:], his[2 * k2][:, ms], wha[:],
                            start=(k2 == 0), stop=False)
                        nc.tensor.matmul(
                            pts[m][:], his[2 * k2 + 1][:, ms], whb[:],
                            start=False, stop=False)
                        if last:
                            nc.tensor.matmul(
                                pts[m][:], los[k2][:, :, ms], wl[:],
                                start=False, stop=True,
                                perf_mode=mybir.MatmulPerfMode.DoubleRow)
                    if not last:
                        for m in range(MT):
                            ms = slice(m * P, (m + 1) * P)
                            nc.tensor.matmul(
                                pts[m][:], los[k2][:, :, ms], wl[:],
                                start=False, stop=False,
                                perf_mode=mybir.MatmulPerfMode.DoubleRow)
                for m in range(MT):
                    ot = op.tile([P, NT], _F32, tag="o", name=f"o{n}_{m}")
                    nc.vector.tensor_copy(ot[:], pts[m][:])
                    nc.sync.dma_start(
                        out[m * P:(m + 1) * P, n * NT:(n + 1) * NT], ot[:])
    nc.compile()
    return nc


def _get_module():
    global _cached
    if _cached is None:
        _cached = _build()
    return _cached


def _run(inputs: np.ndarray, kernel_w: np.ndarray, trace: bool = False):
    nc = _get_module()

    bw = kernel_w > 0.5
    whi = bw.astype(np.float16)
    wlo = (bw.astype(np.float32) / LO_SCALE).astype(ml_dtypes.float8_e4m3)
    hi = inputs.astype(np.float16)
    lo = ((inputs - hi.astype(np.float32)) * LO_SCALE).astype(
        ml_dtypes.float8_e4m3)

    in_maps = []
    for i in range(N_CORES):
        sl = slice(i * TOK, (i + 1) * TOK)
        in_maps.append({
            "xhi": np.ascontiguousarray(hi[sl].T),
            "xlo": np.ascontiguousarray(lo[sl].T),
            "whi": whi,
            "wlo": wlo,
        })

    res = run_bass_kernel_spmd(nc, in_maps, core_ids=list(range(N_CORES)),
                               trace=trace)
    full = np.concatenate([r["out"] for r in res.results], axis=0)
    return full, res


def kernel(inputs: np.ndarray, kernel: np.ndarray) -> np.ndarray:
    return _run(inputs, kernel)[0]
